# revision 4
# baseline (speedup 1.0000x reference)
"""Grouped VQ (EMA codebook) on Trainium2 — 8-core data-parallel SPMD.

Hardcoded problem: x [8,2048,512] f32, embed [4,2048,128], cluster_size
[4,2048], embed_avg [4,2048,128]; G=4, C=2048, dg=128, N=16384.
Core k owns tokens of x[k] (2048). Codebooks replicated; bins/embed_sum
and the commit-loss partial are AllReduced; EMA update replicated.

Per core:
  scores[n,c] = f.(2E)^T - ||E||^2 via one fp32 PE matmul per 512-chunk
  plus a K=2 fp16 matmul folding -(e2_hi+e2_lo) into the same PSUM bank.
  argmax: nc.vector.max/max_index directly on PSUM, over two 1024 halves.
  embed_sum/bins and the quantize gather are one-hot bf16 matmuls
  (x split hi/lo bf16 so sums stay fp32-accurate; E hi/lo for gather).
"""
import sys
sys.path.insert(0, '/opt/trn_rl_repo')
import numpy as np

from concourse import bacc, tile
import concourse.mybir as mybir
from concourse.bass_utils import run_bass_kernel_spmd

F32 = mybir.dt.float32
BF16 = mybir.dt.bfloat16
FP16 = mybir.dt.float16
U32 = mybir.dt.uint32
I32 = mybir.dt.int32
AL = mybir.AluOpType
AF = mybir.ActivationFunctionType

G, C, DG = 4, 2048, 128
NLOC = 2048
NT = NLOC // 128       # 16
CT = C // 128          # 16
GT = G * CT            # 64
NCORES = 8
DECAY = 0.8
EPS = 1e-5

_CACHE = {}


def _build():
    nc = bacc.Bacc("TRN2", target_bir_lowering=False, debug=False,
                   num_devices=NCORES)

    in_x = nc.dram_tensor("in_x", [NLOC, 512], F32, kind="ExternalInput").ap()
    in_e = nc.dram_tensor("in_e", [G, C, DG], F32, kind="ExternalInput").ap()
    in_cs = nc.dram_tensor("in_cs", [128, GT], F32, kind="ExternalInput").ap()
    in_ea = nc.dram_tensor("in_ea", [G, C, DG], F32, kind="ExternalInput").ap()

    out_q = nc.dram_tensor("out_q", [NLOC, 512], F32, kind="ExternalOutput").ap()
    out_ind = nc.dram_tensor("out_ind", [NT, G, 128], I32,
                             kind="ExternalOutput").ap()
    out_ncs = nc.dram_tensor("out_ncs", [GT, 128], F32,
                             kind="ExternalOutput").ap()
    out_nea = nc.dram_tensor("out_nea", [G, C, DG], F32,
                             kind="ExternalOutput").ap()
    out_ne = nc.dram_tensor("out_ne", [G, C, DG], F32,
                            kind="ExternalOutput").ap()
    out_cl = nc.dram_tensor("out_cl", [1, 1], F32, kind="ExternalOutput").ap()

    ar_es_in = nc.dram_tensor("ar_es_in", [GT * 128, DG], F32)
    ar_es_out = nc.dram_tensor("ar_es_out", [GT * 128, DG], F32,
                               addr_space="Shared")
    ar_b_in = nc.dram_tensor("ar_b_in", [128, GT + 1], F32)
    ar_b_out = nc.dram_tensor("ar_b_out", [128, GT + 1], F32,
                              addr_space="Shared")

    id128f = nc.inline_tensor(np.eye(128, dtype=np.float32), name="id128f")
    iota16 = nc.inline_tensor(
        np.tile(np.arange(C, dtype=np.float16), (128, 1)), name="iota16")
    codeio = nc.inline_tensor(
        (np.arange(128, dtype=np.float32)[:, None]
         + 128.0 * np.arange(CT, dtype=np.float32)[None, :]), name="codeio")
    ones2h = nc.inline_tensor(np.ones((2, 128), np.float16), name="ones2h")
    ones1h = nc.inline_tensor(np.ones((1, 128), np.float16), name="ones1h")
    ones1fr = nc.inline_tensor(np.ones((1, 128), np.float32), name="ones1fr")
    ones1fc = nc.inline_tensor(np.ones((128, 1), np.float32), name="ones1fc")

    with tile.TileContext(nc) as tc:
        with (
            tc.tile_pool(name="cst", bufs=1) as cst,
            tc.tile_pool(name="big", bufs=1) as big,
            tc.tile_pool(name="grp", bufs=1) as grp,
            tc.tile_pool(name="wk", bufs=3) as wk,
            tc.tile_pool(name="psA", bufs=2, space="PSUM") as psA,
            tc.tile_pool(name="psB", bufs=1, space="PSUM") as psB,
            tc.tile_pool(name="psC", bufs=1, space="PSUM") as psC,
            tc.tile_pool(name="psM", bufs=2, space="PSUM") as psM,
        ):
            c_idf = cst.tile([128, 128], F32)
            nc.sync.dma_start(c_idf[:], id128f.ap())
            c_iota = cst.tile([128, C], FP16)
            nc.sync.dma_start(c_iota[:], iota16.ap())
            c_cio = cst.tile([128, CT], F32)
            nc.sync.dma_start(c_cio[:], codeio.ap())
            c_o2h = cst.tile([2, 128], FP16)
            nc.sync.dma_start(c_o2h[:], ones2h.ap())
            c_o1h = cst.tile([1, 128], FP16)
            nc.sync.dma_start(c_o1h[:], ones1h.ap())
            c_o1fr = cst.tile([1, 128], F32)
            nc.sync.dma_start(c_o1fr[:], ones1fr.ap())
            c_o1fc = cst.tile([128, 1], F32)
            nc.sync.dma_start(c_o1fc[:], ones1fc.ap())

            t_x = big.tile([128, NT, 512], F32)
            for j in range(NT):
                nc.sync.dma_start(t_x[:, j, :], in_x[j * 128:(j + 1) * 128, :])

            t_ind = big.tile([128, G, NT], F32)        # winning code ids
            t_bins = big.tile([128, GT + 1], F32)      # bins + commit partial
            t_cp = big.tile([128, G * NT], F32)        # commit partials

            for g in range(G):
                # ---- per-group prep ----
                t_eg = grp.tile([128, CT, DG], F32, tag="eg")
                for t in range(CT):
                    nc.sync.dma_start(t_eg[:, t, :],
                                      in_e[g, t * 128:(t + 1) * 128, :])
                # gather rhs [E_hi | E_lo] bf16
                t_ehl = grp.tile([128, CT, 2 * DG], BF16, tag="ehl")
                for t in range(CT):
                    nc.vector.tensor_copy(t_ehl[:, t, 0:DG], t_eg[:, t, :])
                    nc.vector.tensor_tensor(t_ehl[:, t, DG:2 * DG],
                                            t_eg[:, t, :], t_ehl[:, t, 0:DG],
                                            AL.subtract)
                # e2 columns + negate
                t_e2c = grp.tile([128, CT], F32, tag="e2c")
                for t in range(CT):
                    sq = wk.tile([128, DG], F32, tag="sq")
                    nc.scalar.activation(sq[:], t_eg[:, t, :], AF.Square,
                                         accum_out=t_e2c[:, t:t + 1])
                t_e2n = grp.tile([128, CT], F32, tag="e2n")
                nc.vector.tensor_scalar(t_e2n[:], t_e2c[:], -1.0, None, AL.mult)
                # transpose -> rows, fp16 hi/lo, flatten to [2, C]
                pm = psM.tile([128, 128], F32, tag="m")
                nc.tensor.transpose(pm[0:CT, :], t_e2n[:], c_idf[:])
                t_e2nr = grp.tile([CT, 128], F32, tag="e2nr")
                nc.scalar.copy(t_e2nr[:], pm[0:CT, :])
                t_e2h = grp.tile([CT, 128], FP16, tag="e2h")
                t_e2l = grp.tile([CT, 128], FP16, tag="e2l")
                nc.vector.tensor_copy(t_e2h[:], t_e2nr[:])
                nc.vector.tensor_tensor(t_e2l[:], t_e2nr[:], t_e2h[:],
                                        AL.subtract)
                t_e2r = grp.tile([2, C], FP16, tag="e2r")
                for t in range(CT):
                    nc.sync.dma_start(t_e2r[0:1, t * 128:(t + 1) * 128],
                                      t_e2h[t:t + 1, :])
                    nc.sync.dma_start(t_e2r[1:2, t * 128:(t + 1) * 128],
                                      t_e2l[t:t + 1, :])
                # transposes: eT2 = (2E)^T fp32 ; xT_g fp32
                t_eT2 = grp.tile([128, C], F32, tag="eT2")
                for t in range(CT):
                    pm = psM.tile([128, 128], F32, tag="m")
                    nc.tensor.transpose(pm[:], t_eg[:, t, :], c_idf[:])
                    nc.scalar.mul(t_eT2[:, t * 128:(t + 1) * 128], pm[:], 2.0)
                t_xT = grp.tile([128, NLOC], F32, tag="xT")
                for j in range(NT):
                    pm = psM.tile([128, 128], F32, tag="m")
                    nc.tensor.transpose(pm[:], t_x[:, j, g * 128:(g + 1) * 128],
                                        c_idf[:])
                    nc.scalar.copy(t_xT[:, j * 128:(j + 1) * 128], pm[:])
                # embed_sum rhs [x_hi | x_lo | 1] bf16 per token tile
                t_rhs = grp.tile([128, NT, 2 * DG + 1], BF16, tag="rhs")
                nc.vector.memset(t_rhs[:, :, 2 * DG], 1.0)
                for j in range(NT):
                    nc.vector.tensor_copy(t_rhs[:, j, 0:DG],
                                          t_x[:, j, g * 128:(g + 1) * 128])
                    nc.vector.tensor_tensor(t_rhs[:, j, DG:2 * DG],
                                            t_x[:, j, g * 128:(g + 1) * 128],
                                            t_rhs[:, j, 0:DG], AL.subtract)

                # ---- dist + argmax ----
                t_mx = grp.tile([128, NT, 2, 8], F32, tag="mx")
                t_ix = grp.tile([128, NT, 2, 8], U32, tag="ix")
                for j in range(NT):
                    for h in range(2):
                        pd = psA.tile([128, 1024], F32, tag="dist")
                        for q in range(2):
                            cc = h * 2 + q
                            nc.tensor.matmul(
                                pd[:, q * 512:(q + 1) * 512],
                                t_xT[:, j * 128:(j + 1) * 128],
                                t_eT2[:, cc * 512:(cc + 1) * 512],
                                start=True, stop=False)
                            nc.tensor.matmul(
                                pd[:, q * 512:(q + 1) * 512],
                                c_o2h[:],
                                t_e2r[:, cc * 512:(cc + 1) * 512],
                                start=False, stop=True)
                        nc.vector.max(t_mx[:, j, h, :], pd[:])
                        nc.vector.max_index(t_ix[:, j, h, :], t_mx[:, j, h, :],
                                            pd[:])
                # combine halves (vectorized over NT)
                t_sel = wk.tile([128, NT], F32, tag="sel")
                nc.vector.tensor_tensor(t_sel[:], t_mx[:, :, 0, 0],
                                        t_mx[:, :, 1, 0], AL.is_ge)
                t_i0 = wk.tile([128, NT], F32, tag="i0")
                nc.vector.tensor_copy(t_i0[:], t_ix[:, :, 0, 0])
                t_i1 = wk.tile([128, NT], F32, tag="i1")
                nc.vector.tensor_copy(t_i1[:], t_ix[:, :, 1, 0])
                nc.vector.tensor_scalar(t_i1[:], t_i1[:], 1024.0, None, AL.add)
                t_d = wk.tile([128, NT], F32, tag="idd")
                nc.vector.tensor_tensor(t_d[:], t_i0[:], t_i1[:], AL.subtract)
                nc.vector.tensor_tensor(t_d[:], t_sel[:], t_d[:], AL.mult)
                nc.vector.tensor_tensor(t_ind[:, g, :], t_i1[:], t_d[:], AL.add)

                # ---- ind rows: transpose, int32 out, fp16 row, broadcast ----
                pm = psM.tile([128, 128], F32, tag="m")
                nc.tensor.transpose(pm[0:NT, :], t_ind[:, g, :], c_idf[:])
                t_indi = wk.tile([NT, 128], I32, tag="indi")
                nc.scalar.copy(t_indi[:], pm[0:NT, :])
                nc.sync.dma_start(out_ind[:, g, :], t_indi[:])
                t_indr = wk.tile([NT, 128], FP16, tag="indr")
                nc.scalar.copy(t_indr[:], pm[0:NT, :])
                t_irow = grp.tile([1, NLOC], FP16, tag="irow")
                for j in range(NT):
                    nc.sync.dma_start(t_irow[0:1, j * 128:(j + 1) * 128],
                                      t_indr[j:j + 1, :])
                t_ibc = grp.tile([128, NLOC], FP16, tag="ibc")
                for j in range(NT):
                    pm = psM.tile([128, 128], F32, tag="m")
                    nc.tensor.matmul(pm[:], c_o1h[:],
                                     t_irow[0:1, j * 128:(j + 1) * 128],
                                     start=True, stop=True)
                    nc.scalar.copy(t_ibc[:, j * 128:(j + 1) * 128], pm[:])

                # ---- onehot [n|c] -> embed_sum + bins ----
                t_oh = big.tile([128, NT, C], BF16, tag="oh")
                for j in range(NT):
                    nc.vector.tensor_scalar(t_oh[:, j, :], c_iota[:],
                                            t_ind[:, g, j:j + 1], None,
                                            AL.is_equal)
                for t in range(CT):
                    pe = psB.tile([128, 2 * DG + 1], F32, tag="es")
                    for j in range(NT):
                        nc.tensor.matmul(pe[:],
                                         t_oh[:, j, t * 128:(t + 1) * 128],
                                         t_rhs[:, j, :],
                                         start=(j == 0), stop=(j == NT - 1))
                    esh = wk.tile([128, DG], F32, tag="esh")
                    nc.scalar.copy(esh[:], pe[:, 0:DG])
                    es = wk.tile([128, DG], F32, tag="esd")
                    nc.vector.tensor_tensor(es[:], pe[:, DG:2 * DG], esh[:],
                                            AL.add)
                    nc.sync.dma_start(
                        ar_es_in.ap()[(g * CT + t) * 128:(g * CT + t + 1) * 128, :],
                        es[:])
                    nc.scalar.copy(t_bins[:, g * CT + t:g * CT + t + 1],
                                   pe[:, 2 * DG:2 * DG + 1])

                # ---- onehotT [c|n] -> gather quantize ----
                t_ohT = big.tile([128, CT, NLOC], BF16, tag="oh")
                for t in range(CT):
                    nc.vector.tensor_scalar(t_ohT[:, t, :], t_ibc[:],
                                            c_cio[:, t:t + 1], None,
                                            AL.is_equal)
                for j in range(NT):
                    pq = psC.tile([128, 2 * DG], F32, tag="q")
                    for t in range(CT):
                        nc.tensor.matmul(pq[:],
                                         t_ohT[:, t, j * 128:(j + 1) * 128],
                                         t_ehl[:, t, :],
                                         start=(t == 0), stop=(t == CT - 1))
                    tqh = wk.tile([128, DG], F32, tag="tqh")
                    nc.scalar.copy(tqh[:], pq[:, 0:DG])
                    tq = wk.tile([128, DG], F32, tag="tq")
                    nc.vector.tensor_tensor(tq[:], pq[:, DG:2 * DG], tqh[:],
                                            AL.add)
                    nc.sync.dma_start(
                        out_q[j * 128:(j + 1) * 128, g * 128:(g + 1) * 128],
                        tq[:])
                    td = wk.tile([128, DG], F32, tag="td")
                    nc.vector.tensor_tensor(td[:], tq[:],
                                            t_x[:, j, g * 128:(g + 1) * 128],
                                            AL.subtract)
                    nc.scalar.activation(td[:], td[:], AF.Square,
                                         accum_out=t_cp[:, g * NT + j:g * NT + j + 1])

            # ---- commit partial col + AllReduce ----
            nc.vector.tensor_reduce(t_bins[:, GT:GT + 1], t_cp[:],
                                    mybir.AxisListType.X, AL.add)
            nc.sync.dma_start(ar_b_in.ap(), t_bins[:])
            nc.gpsimd.collective_compute(
                "AllReduce", AL.add, replica_groups=[list(range(NCORES))],
                ins=[ar_b_in.ap().opt()], outs=[ar_b_out.ap().opt()])
            nc.gpsimd.collective_compute(
                "AllReduce", AL.add, replica_groups=[list(range(NCORES))],
                ins=[ar_es_in.ap().opt()], outs=[ar_es_out.ap().opt()])

            # ---- EMA update (replicated on every core) ----
            t_bar = big.tile([128, GT + 1], F32, tag="bar")
            nc.sync.dma_start(t_bar[:], ar_b_out.ap())
            t_cs = big.tile([128, GT], F32, tag="cs")
            nc.sync.dma_start(t_cs[:], in_cs[:])
            t_ncs = big.tile([128, GT], F32, tag="ncs")
            t_tmp = wk.tile([128, GT], F32, tag="tmp")
            nc.vector.tensor_scalar(t_tmp[:], t_cs[:], DECAY, None, AL.mult)
            nc.vector.scalar_tensor_tensor(t_ncs[:], t_bar[:, 0:GT],
                                           1.0 - DECAY, t_tmp[:],
                                           AL.mult, AL.add)
            # partition sums (ncs totals + global commit)
            pm = psM.tile([128, 128], F32, tag="m")
            nc.tensor.matmul(pm[0:1, 0:GT], c_o1fc[:], t_ncs[:],
                             start=True, stop=True)
            nc.tensor.matmul(pm[0:1, GT:GT + 1], c_o1fc[:],
                             t_bar[:, GT:GT + 1], start=True, stop=True)
            s_su = wk.tile([1, GT + 1], F32, tag="ssu")
            nc.scalar.copy(s_su[:], pm[0:1, 0:GT + 1])
            s_cl = wk.tile([1, 1], F32, tag="scl")
            nc.vector.tensor_scalar(s_cl[:], s_su[:, GT:GT + 1],
                                    1.0 / (NCORES * NLOC * 512), None, AL.mult)
            nc.sync.dma_start(out_cl[:], s_cl[:])
            t_tot = wk.tile([1, G], F32, tag="tot")
            for g in range(G):
                nc.vector.tensor_reduce(t_tot[:, g:g + 1],
                                        s_su[:, g * CT:(g + 1) * CT],
                                        mybir.AxisListType.X, AL.add)
            t_totx = wk.tile([1, GT], F32, tag="totx")
            for g in range(G):
                nc.vector.tensor_scalar(t_totx[:, g * CT:(g + 1) * CT],
                                        c_o1fr[0:1, 0:CT], t_tot[:, g:g + 1],
                                        None, AL.mult)
            pm2 = psM.tile([128, 128], F32, tag="m")
            nc.tensor.matmul(pm2[:, 0:GT], c_o1fr[:], t_totx[:],
                             start=True, stop=True)
            t_totb = wk.tile([128, GT], F32, tag="totb")
            nc.scalar.copy(t_totb[:], pm2[:, 0:GT])
            # smoothed + reciprocal
            t_sm = wk.tile([128, GT], F32, tag="sm")
            nc.vector.tensor_scalar(t_sm[:], t_ncs[:], EPS, None, AL.add)
            t_den = wk.tile([128, GT], F32, tag="den")
            nc.vector.tensor_scalar(t_den[:], t_totb[:], C * EPS, None, AL.add)
            t_rden = wk.tile([128, GT], F32, tag="rden")
            nc.vector.reciprocal(t_rden[:], t_den[:])
            nc.vector.tensor_tensor(t_sm[:], t_sm[:], t_rden[:], AL.mult)
            nc.vector.tensor_tensor(t_sm[:], t_sm[:], t_totb[:], AL.mult)
            t_rsm = wk.tile([128, GT], F32, tag="rsm")
            nc.vector.reciprocal(t_rsm[:], t_sm[:])
            # ncs out (transposed)
            pm3 = psM.tile([128, 128], F32, tag="m")
            nc.tensor.transpose(pm3[0:GT, :], t_ncs[:], c_idf[:])
            s_nc = wk.tile([GT, 128], F32, tag="snc")
            nc.scalar.copy(s_nc[:], pm3[0:GT, :])
            nc.sync.dma_start(out_ncs[:], s_nc[:])
            # nea / ne
            for g in range(G):
                for t in range(CT):
                    ea = wk.tile([128, DG], F32, tag="ea")
                    nc.sync.dma_start(ea[:], in_ea[g, t * 128:(t + 1) * 128, :])
                    esr = wk.tile([128, DG], F32, tag="esr")
                    nc.sync.dma_start(
                        esr[:],
                        ar_es_out.ap()[(g * CT + t) * 128:(g * CT + t + 1) * 128, :])
                    nea = wk.tile([128, DG], F32, tag="nea")
                    nc.vector.tensor_scalar(nea[:], ea[:], DECAY, None, AL.mult)
                    nc.vector.scalar_tensor_tensor(nea[:], esr[:], 1.0 - DECAY,
                                                   nea[:], AL.mult, AL.add)
                    nc.sync.dma_start(out_nea[g, t * 128:(t + 1) * 128, :],
                                      nea[:])
                    ne = wk.tile([128, DG], F32, tag="ne")
                    nc.vector.tensor_scalar(
                        ne[:], nea[:], t_rsm[:, g * CT + t:g * CT + t + 1],
                        None, AL.mult)
                    nc.sync.dma_start(out_ne[g, t * 128:(t + 1) * 128, :],
                                      ne[:])

    nc.compile()
    return nc


def _get_nc():
    if "nc" not in _CACHE:
        _CACHE["nc"] = _build()
    return _CACHE["nc"]


def run(x, embed, cluster_size, embed_avg, trace=False, tmpdir=None):
    nc = _get_nc()
    x = np.ascontiguousarray(np.asarray(x, dtype=np.float32))
    embed = np.ascontiguousarray(np.asarray(embed, dtype=np.float32))
    cluster_size = np.asarray(cluster_size, dtype=np.float32)
    embed_avg = np.ascontiguousarray(np.asarray(embed_avg, dtype=np.float32))
    # cluster_size -> [128 c-local, 64 (g,t)] layout
    cs_t = np.ascontiguousarray(
        cluster_size.reshape(G, CT, 128).transpose(2, 0, 1).reshape(128, GT))
    in_maps = [
        {"in_x": np.ascontiguousarray(x[i]), "in_e": embed, "in_cs": cs_t,
         "in_ea": embed_avg}
        for i in range(NCORES)
    ]
    res = run_bass_kernel_spmd(nc, in_maps, list(range(NCORES)),
                               trace=trace, tmpdir=tmpdir)
    r0 = res.results[0]
    q = np.stack([res.results[i]["out_q"] for i in range(NCORES)])
    ind = np.stack([
        res.results[i]["out_ind"].transpose(0, 2, 1).reshape(NLOC, G)
        for i in range(NCORES)
    ]).astype(np.int32)
    commit = np.float32(r0["out_cl"][0, 0])
    ncs = r0["out_ncs"].reshape(G, CT, 128).reshape(G, C)
    nea = r0["out_nea"]
    ne = r0["out_ne"]
    return (q, ind, commit, ncs, nea, ne), res


def kernel(x, embed, cluster_size, embed_avg):
    out, _ = run(x, embed, cluster_size, embed_avg, trace=False)
    return out


# revision 6
# speedup vs baseline: 1.0086x; 1.0086x over previous
"""Grouped VQ (EMA codebook) on Trainium2 — 8-core data-parallel SPMD.

Hardcoded problem: x [8,2048,512] f32, embed [4,2048,128], cluster_size
[4,2048], embed_avg [4,2048,128]; G=4, C=2048, dg=128, N=16384.
Core k owns tokens of x[k] (2048). Codebooks replicated; bins/embed_sum
and the commit-loss partial are AllReduced; EMA update replicated.

Per core:
  scores[n,c] = f.(2E)^T - ||E||^2 via one fp32 PE matmul per 512-chunk
  plus a K=2 fp16 matmul folding -(e2_hi+e2_lo) into the same PSUM bank.
  argmax: nc.vector.max/max_index directly on PSUM, over two 1024 halves.
  embed_sum/bins and the quantize gather are one-hot bf16 matmuls
  (x split hi/lo bf16 so sums stay fp32-accurate; E hi/lo for gather).
"""
import sys
sys.path.insert(0, '/opt/trn_rl_repo')
import numpy as np

from concourse import bacc, tile
import concourse.mybir as mybir
from concourse.bass_utils import run_bass_kernel_spmd

F32 = mybir.dt.float32
BF16 = mybir.dt.bfloat16
FP16 = mybir.dt.float16
U32 = mybir.dt.uint32
I32 = mybir.dt.int32
AL = mybir.AluOpType
AF = mybir.ActivationFunctionType

G, C, DG = 4, 2048, 128
NLOC = 2048
NT = NLOC // 128       # 16
CT = C // 128          # 16
GT = G * CT            # 64
NCORES = 8
DECAY = 0.8
EPS = 1e-5

_CACHE = {}


def _build():
    nc = bacc.Bacc("TRN2", target_bir_lowering=False, debug=False,
                   num_devices=NCORES)

    in_x = nc.dram_tensor("in_x", [NLOC, 512], F32, kind="ExternalInput").ap()
    in_e = nc.dram_tensor("in_e", [G, C, DG], F32, kind="ExternalInput").ap()
    in_cs = nc.dram_tensor("in_cs", [128, GT], F32, kind="ExternalInput").ap()
    in_ea = nc.dram_tensor("in_ea", [G, C, DG], F32, kind="ExternalInput").ap()

    out_q = nc.dram_tensor("out_q", [NLOC, 512], F32, kind="ExternalOutput").ap()
    out_ind = nc.dram_tensor("out_ind", [NT, G, 128], I32,
                             kind="ExternalOutput").ap()
    out_ncs = nc.dram_tensor("out_ncs", [GT, 128], F32,
                             kind="ExternalOutput").ap()
    out_nea = nc.dram_tensor("out_nea", [G, C, DG], F32,
                             kind="ExternalOutput").ap()
    out_ne = nc.dram_tensor("out_ne", [G, C, DG], F32,
                            kind="ExternalOutput").ap()
    out_cl = nc.dram_tensor("out_cl", [1, 1], F32, kind="ExternalOutput").ap()

    ar_es_in = nc.dram_tensor("ar_es_in", [GT * 128, DG], F32)
    ar_es_out = nc.dram_tensor("ar_es_out", [GT * 128, DG], F32,
                               addr_space="Shared")
    ar_b_in = nc.dram_tensor("ar_b_in", [128, GT + 1], F32)
    ar_b_out = nc.dram_tensor("ar_b_out", [128, GT + 1], F32,
                              addr_space="Shared")

    id128f = nc.inline_tensor(np.eye(128, dtype=np.float32), name="id128f")
    iota16 = nc.inline_tensor(
        np.tile(np.arange(C, dtype=np.float16), (128, 1)), name="iota16")
    codeio = nc.inline_tensor(
        (np.arange(128, dtype=np.float32)[:, None]
         + 128.0 * np.arange(CT, dtype=np.float32)[None, :]), name="codeio")
    ones2h = nc.inline_tensor(np.ones((2, 128), np.float16), name="ones2h")
    ones1h = nc.inline_tensor(np.ones((1, 128), np.float16), name="ones1h")
    ones1fr = nc.inline_tensor(np.ones((1, 128), np.float32), name="ones1fr")
    ones1fc = nc.inline_tensor(np.ones((128, 1), np.float32), name="ones1fc")

    with tile.TileContext(nc) as tc:
        with (
            tc.tile_pool(name="cst", bufs=1) as cst,
            tc.tile_pool(name="big", bufs=1) as big,
            tc.tile_pool(name="grp", bufs=1) as grp,
            tc.tile_pool(name="grp2", bufs=2) as grp2,
            tc.tile_pool(name="wk", bufs=2) as wk,
            tc.tile_pool(name="psA", bufs=2, space="PSUM") as psA,
            tc.tile_pool(name="psB", bufs=1, space="PSUM") as psB,
            tc.tile_pool(name="psC", bufs=1, space="PSUM") as psC,
            tc.tile_pool(name="psM", bufs=2, space="PSUM") as psM,
        ):
            c_idf = cst.tile([128, 128], F32)
            nc.sync.dma_start(c_idf[:], id128f.ap())
            c_iota = cst.tile([128, C], FP16)
            nc.sync.dma_start(c_iota[:], iota16.ap())
            c_cio = cst.tile([128, CT], F32)
            nc.sync.dma_start(c_cio[:], codeio.ap())
            c_o2h = cst.tile([2, 128], FP16)
            nc.sync.dma_start(c_o2h[:], ones2h.ap())
            c_o1h = cst.tile([1, 128], FP16)
            nc.sync.dma_start(c_o1h[:], ones1h.ap())
            c_o1fr = cst.tile([1, 128], F32)
            nc.sync.dma_start(c_o1fr[:], ones1fr.ap())
            c_o1fc = cst.tile([128, 1], F32)
            nc.sync.dma_start(c_o1fc[:], ones1fc.ap())

            t_x = big.tile([128, NT, 512], F32)
            for j in range(NT):
                nc.sync.dma_start(t_x[:, j, :], in_x[j * 128:(j + 1) * 128, :])

            t_ind = big.tile([128, G, NT], F32)        # winning code ids
            t_bins = big.tile([128, GT + 1], F32)      # bins + commit partial
            t_cp = big.tile([128, G * NT], F32)        # commit partials

            def prep_group(g):
                t_eg = grp2.tile([128, CT, DG], F32, tag="eg")
                for t in range(CT):
                    nc.sync.dma_start(t_eg[:, t, :],
                                      in_e[g, t * 128:(t + 1) * 128, :])
                # gather rhs [E_hi | E_lo] bf16
                t_ehl = grp.tile([128, CT, 2 * DG], BF16, tag="ehl")
                for t in range(CT):
                    nc.vector.tensor_copy(t_ehl[:, t, 0:DG], t_eg[:, t, :])
                    nc.vector.tensor_tensor(t_ehl[:, t, DG:2 * DG],
                                            t_eg[:, t, :], t_ehl[:, t, 0:DG],
                                            AL.subtract)
                # e2 columns + negate
                t_e2c = grp2.tile([128, CT], F32, tag="e2c")
                for t in range(CT):
                    sq = wk.tile([128, DG], F32, tag="sq")
                    nc.scalar.activation(sq[:], t_eg[:, t, :], AF.Square,
                                         accum_out=t_e2c[:, t:t + 1])
                t_e2n = grp2.tile([128, CT], F32, tag="e2n")
                nc.vector.tensor_scalar(t_e2n[:], t_e2c[:], -1.0, None, AL.mult)
                # transpose -> rows, fp16 hi/lo, flatten to [2, C]
                pm = psM.tile([128, 128], F32, tag="m")
                nc.tensor.transpose(pm[0:CT, :], t_e2n[:], c_idf[:])
                t_e2nr = grp2.tile([CT, 128], F32, tag="e2nr")
                nc.scalar.copy(t_e2nr[:], pm[0:CT, :])
                t_e2h = grp2.tile([CT, 128], FP16, tag="e2h")
                t_e2l = grp2.tile([CT, 128], FP16, tag="e2l")
                nc.vector.tensor_copy(t_e2h[:], t_e2nr[:])
                nc.vector.tensor_tensor(t_e2l[:], t_e2nr[:], t_e2h[:],
                                        AL.subtract)
                t_e2r = grp2.tile([2, C], FP16, tag="e2r")
                for t in range(CT):
                    nc.sync.dma_start(t_e2r[0:1, t * 128:(t + 1) * 128],
                                      t_e2h[t:t + 1, :])
                    nc.sync.dma_start(t_e2r[1:2, t * 128:(t + 1) * 128],
                                      t_e2l[t:t + 1, :])
                # transposes: eT2 = (2E)^T fp32 ; xT_g fp32
                t_eT2 = grp2.tile([128, C], F32, tag="eT2")
                for t in range(CT):
                    pm = psM.tile([128, 128], F32, tag="m")
                    nc.tensor.transpose(pm[:], t_eg[:, t, :], c_idf[:])
                    nc.scalar.mul(t_eT2[:, t * 128:(t + 1) * 128], pm[:], 2.0)
                t_xT = grp2.tile([128, NLOC], F32, tag="xT")
                for j in range(NT):
                    pm = psM.tile([128, 128], F32, tag="m")
                    nc.tensor.transpose(pm[:], t_x[:, j, g * 128:(g + 1) * 128],
                                        c_idf[:])
                    nc.scalar.copy(t_xT[:, j * 128:(j + 1) * 128], pm[:])
                # embed_sum rhs [x_hi | x_lo | 1] bf16 per token tile
                t_rhs = grp.tile([128, NT, 2 * DG + 1], BF16, tag="rhs")
                nc.vector.memset(t_rhs[:, :, 2 * DG], 1.0)
                for j in range(NT):
                    nc.vector.tensor_copy(t_rhs[:, j, 0:DG],
                                          t_x[:, j, g * 128:(g + 1) * 128])
                    nc.vector.tensor_tensor(t_rhs[:, j, DG:2 * DG],
                                            t_x[:, j, g * 128:(g + 1) * 128],
                                            t_rhs[:, j, 0:DG], AL.subtract)
                return t_ehl, t_e2r, t_eT2, t_xT, t_rhs

            nxt = prep_group(0)
            for g in range(G):
                t_ehl, t_e2r, t_eT2, t_xT, t_rhs = nxt

                # ---- dist + argmax ----
                t_mx = grp.tile([128, NT, 2, 8], F32, tag="mx")
                t_ix = grp.tile([128, NT, 2, 8], U32, tag="ix")
                for j in range(NT):
                    for h in range(2):
                        pd = psA.tile([128, 1024], F32, tag="dist")
                        for q in range(2):
                            cc = h * 2 + q
                            nc.tensor.matmul(
                                pd[:, q * 512:(q + 1) * 512],
                                t_xT[:, j * 128:(j + 1) * 128],
                                t_eT2[:, cc * 512:(cc + 1) * 512],
                                start=True, stop=False)
                            nc.tensor.matmul(
                                pd[:, q * 512:(q + 1) * 512],
                                c_o2h[:],
                                t_e2r[:, cc * 512:(cc + 1) * 512],
                                start=False, stop=True)
                        nc.vector.max(t_mx[:, j, h, :], pd[:])
                        nc.vector.max_index(t_ix[:, j, h, :], t_mx[:, j, h, :],
                                            pd[:])
                # combine halves (vectorized over NT)
                t_sel = wk.tile([128, NT], F32, tag="sel")
                nc.vector.tensor_tensor(t_sel[:], t_mx[:, :, 0, 0],
                                        t_mx[:, :, 1, 0], AL.is_ge)
                t_i0 = wk.tile([128, NT], F32, tag="i0")
                nc.vector.tensor_copy(t_i0[:], t_ix[:, :, 0, 0])
                t_i1 = wk.tile([128, NT], F32, tag="i1")
                nc.vector.tensor_copy(t_i1[:], t_ix[:, :, 1, 0])
                nc.vector.tensor_scalar(t_i1[:], t_i1[:], 1024.0, None, AL.add)
                t_d = wk.tile([128, NT], F32, tag="idd")
                nc.vector.tensor_tensor(t_d[:], t_i0[:], t_i1[:], AL.subtract)
                nc.vector.tensor_tensor(t_d[:], t_sel[:], t_d[:], AL.mult)
                nc.vector.tensor_tensor(t_ind[:, g, :], t_i1[:], t_d[:], AL.add)

                if g + 1 < G:
                    nxt = prep_group(g + 1)

                # ---- ind rows: transpose, int32 out, fp16 row, broadcast ----
                pm = psM.tile([128, 128], F32, tag="m")
                nc.tensor.transpose(pm[0:NT, :], t_ind[:, g, :], c_idf[:])
                t_indi = wk.tile([NT, 128], I32, tag="indi")
                nc.scalar.copy(t_indi[:], pm[0:NT, :])
                nc.sync.dma_start(out_ind[:, g, :], t_indi[:])
                t_indr = wk.tile([NT, 128], FP16, tag="indr")
                nc.scalar.copy(t_indr[:], pm[0:NT, :])
                t_irow = grp.tile([1, NLOC], FP16, tag="irow")
                for j in range(NT):
                    nc.sync.dma_start(t_irow[0:1, j * 128:(j + 1) * 128],
                                      t_indr[j:j + 1, :])
                t_ibc = grp.tile([128, NLOC], FP16, tag="ibc")
                for j in range(NT):
                    pm = psM.tile([128, 128], F32, tag="m")
                    nc.tensor.matmul(pm[:], c_o1h[:],
                                     t_irow[0:1, j * 128:(j + 1) * 128],
                                     start=True, stop=True)
                    nc.scalar.copy(t_ibc[:, j * 128:(j + 1) * 128], pm[:])

                # ---- onehot [n|c] -> embed_sum + bins ----
                t_oh = big.tile([128, NT, C], BF16, tag="oh")
                for j in range(NT):
                    nc.vector.tensor_scalar(t_oh[:, j, :], c_iota[:],
                                            t_ind[:, g, j:j + 1], None,
                                            AL.is_equal)
                for t in range(CT):
                    pe = psB.tile([128, 2 * DG + 1], F32, tag="es")
                    for j in range(NT):
                        nc.tensor.matmul(pe[:],
                                         t_oh[:, j, t * 128:(t + 1) * 128],
                                         t_rhs[:, j, :],
                                         start=(j == 0), stop=(j == NT - 1))
                    esh = wk.tile([128, DG], F32, tag="esh")
                    nc.scalar.copy(esh[:], pe[:, 0:DG])
                    es = wk.tile([128, DG], F32, tag="esd")
                    nc.vector.tensor_tensor(es[:], pe[:, DG:2 * DG], esh[:],
                                            AL.add)
                    nc.sync.dma_start(
                        ar_es_in.ap()[(g * CT + t) * 128:(g * CT + t + 1) * 128, :],
                        es[:])
                    nc.scalar.copy(t_bins[:, g * CT + t:g * CT + t + 1],
                                   pe[:, 2 * DG:2 * DG + 1])

                # ---- onehotT [c|n] -> gather quantize ----
                t_ohT = big.tile([128, CT, NLOC], BF16, tag="oh")
                for t in range(CT):
                    nc.vector.tensor_scalar(t_ohT[:, t, :], t_ibc[:],
                                            c_cio[:, t:t + 1], None,
                                            AL.is_equal)
                for j in range(NT):
                    pq = psC.tile([128, 2 * DG], F32, tag="q")
                    for t in range(CT):
                        nc.tensor.matmul(pq[:],
                                         t_ohT[:, t, j * 128:(j + 1) * 128],
                                         t_ehl[:, t, :],
                                         start=(t == 0), stop=(t == CT - 1))
                    tqh = wk.tile([128, DG], F32, tag="tqh")
                    nc.scalar.copy(tqh[:], pq[:, 0:DG])
                    tq = wk.tile([128, DG], F32, tag="tq")
                    nc.vector.tensor_tensor(tq[:], pq[:, DG:2 * DG], tqh[:],
                                            AL.add)
                    nc.sync.dma_start(
                        out_q[j * 128:(j + 1) * 128, g * 128:(g + 1) * 128],
                        tq[:])
                    td = wk.tile([128, DG], F32, tag="td")
                    nc.vector.tensor_tensor(td[:], tq[:],
                                            t_x[:, j, g * 128:(g + 1) * 128],
                                            AL.subtract)
                    nc.scalar.activation(td[:], td[:], AF.Square,
                                         accum_out=t_cp[:, g * NT + j:g * NT + j + 1])

            # ---- commit partial col + AllReduce ----
            nc.vector.tensor_reduce(t_bins[:, GT:GT + 1], t_cp[:],
                                    mybir.AxisListType.X, AL.add)
            nc.sync.dma_start(ar_b_in.ap(), t_bins[:])
            nc.gpsimd.collective_compute(
                "AllReduce", AL.add, replica_groups=[list(range(NCORES))],
                ins=[ar_b_in.ap().opt()], outs=[ar_b_out.ap().opt()])
            nc.gpsimd.collective_compute(
                "AllReduce", AL.add, replica_groups=[list(range(NCORES))],
                ins=[ar_es_in.ap().opt()], outs=[ar_es_out.ap().opt()])

            # ---- EMA update (replicated on every core) ----
            t_bar = big.tile([128, GT + 1], F32, tag="bar")
            nc.sync.dma_start(t_bar[:], ar_b_out.ap())
            t_cs = big.tile([128, GT], F32, tag="cs")
            nc.sync.dma_start(t_cs[:], in_cs[:])
            t_ncs = big.tile([128, GT], F32, tag="ncs")
            t_tmp = wk.tile([128, GT], F32, tag="tmp")
            nc.vector.tensor_scalar(t_tmp[:], t_cs[:], DECAY, None, AL.mult)
            nc.vector.scalar_tensor_tensor(t_ncs[:], t_bar[:, 0:GT],
                                           1.0 - DECAY, t_tmp[:],
                                           AL.mult, AL.add)
            # partition sums (ncs totals + global commit)
            pm = psM.tile([128, 128], F32, tag="m")
            nc.tensor.matmul(pm[0:1, 0:GT], c_o1fc[:], t_ncs[:],
                             start=True, stop=True)
            nc.tensor.matmul(pm[0:1, GT:GT + 1], c_o1fc[:],
                             t_bar[:, GT:GT + 1], start=True, stop=True)
            s_su = wk.tile([1, GT + 1], F32, tag="ssu")
            nc.scalar.copy(s_su[:], pm[0:1, 0:GT + 1])
            s_cl = wk.tile([1, 1], F32, tag="scl")
            nc.vector.tensor_scalar(s_cl[:], s_su[:, GT:GT + 1],
                                    1.0 / (NCORES * NLOC * 512), None, AL.mult)
            nc.sync.dma_start(out_cl[:], s_cl[:])
            t_tot = wk.tile([1, G], F32, tag="tot")
            for g in range(G):
                nc.vector.tensor_reduce(t_tot[:, g:g + 1],
                                        s_su[:, g * CT:(g + 1) * CT],
                                        mybir.AxisListType.X, AL.add)
            t_totx = wk.tile([1, GT], F32, tag="totx")
            for g in range(G):
                nc.vector.tensor_scalar(t_totx[:, g * CT:(g + 1) * CT],
                                        c_o1fr[0:1, 0:CT], t_tot[:, g:g + 1],
                                        None, AL.mult)
            pm2 = psM.tile([128, 128], F32, tag="m")
            nc.tensor.matmul(pm2[:, 0:GT], c_o1fr[:], t_totx[:],
                             start=True, stop=True)
            t_totb = wk.tile([128, GT], F32, tag="totb")
            nc.scalar.copy(t_totb[:], pm2[:, 0:GT])
            # smoothed + reciprocal
            t_sm = wk.tile([128, GT], F32, tag="sm")
            nc.vector.tensor_scalar(t_sm[:], t_ncs[:], EPS, None, AL.add)
            t_den = wk.tile([128, GT], F32, tag="den")
            nc.vector.tensor_scalar(t_den[:], t_totb[:], C * EPS, None, AL.add)
            t_rden = wk.tile([128, GT], F32, tag="rden")
            nc.vector.reciprocal(t_rden[:], t_den[:])
            nc.vector.tensor_tensor(t_sm[:], t_sm[:], t_rden[:], AL.mult)
            nc.vector.tensor_tensor(t_sm[:], t_sm[:], t_totb[:], AL.mult)
            t_rsm = wk.tile([128, GT], F32, tag="rsm")
            nc.vector.reciprocal(t_rsm[:], t_sm[:])
            # ncs out (transposed)
            pm3 = psM.tile([128, 128], F32, tag="m")
            nc.tensor.transpose(pm3[0:GT, :], t_ncs[:], c_idf[:])
            s_nc = wk.tile([GT, 128], F32, tag="snc")
            nc.scalar.copy(s_nc[:], pm3[0:GT, :])
            nc.sync.dma_start(out_ncs[:], s_nc[:])
            # nea / ne
            for g in range(G):
                for t in range(CT):
                    ea = wk.tile([128, DG], F32, tag="ea")
                    nc.sync.dma_start(ea[:], in_ea[g, t * 128:(t + 1) * 128, :])
                    esr = wk.tile([128, DG], F32, tag="esr")
                    nc.sync.dma_start(
                        esr[:],
                        ar_es_out.ap()[(g * CT + t) * 128:(g * CT + t + 1) * 128, :])
                    nea = wk.tile([128, DG], F32, tag="nea")
                    nc.vector.tensor_scalar(nea[:], ea[:], DECAY, None, AL.mult)
                    nc.vector.scalar_tensor_tensor(nea[:], esr[:], 1.0 - DECAY,
                                                   nea[:], AL.mult, AL.add)
                    nc.sync.dma_start(out_nea[g, t * 128:(t + 1) * 128, :],
                                      nea[:])
                    ne = wk.tile([128, DG], F32, tag="ne")
                    nc.vector.tensor_scalar(
                        ne[:], nea[:], t_rsm[:, g * CT + t:g * CT + t + 1],
                        None, AL.mult)
                    nc.sync.dma_start(out_ne[g, t * 128:(t + 1) * 128, :],
                                      ne[:])

    nc.compile()
    return nc


def _get_nc():
    if "nc" not in _CACHE:
        _CACHE["nc"] = _build()
    return _CACHE["nc"]


def run(x, embed, cluster_size, embed_avg, trace=False, tmpdir=None):
    nc = _get_nc()
    x = np.ascontiguousarray(np.asarray(x, dtype=np.float32))
    embed = np.ascontiguousarray(np.asarray(embed, dtype=np.float32))
    cluster_size = np.asarray(cluster_size, dtype=np.float32)
    embed_avg = np.ascontiguousarray(np.asarray(embed_avg, dtype=np.float32))
    # cluster_size -> [128 c-local, 64 (g,t)] layout
    cs_t = np.ascontiguousarray(
        cluster_size.reshape(G, CT, 128).transpose(2, 0, 1).reshape(128, GT))
    in_maps = [
        {"in_x": np.ascontiguousarray(x[i]), "in_e": embed, "in_cs": cs_t,
         "in_ea": embed_avg}
        for i in range(NCORES)
    ]
    res = run_bass_kernel_spmd(nc, in_maps, list(range(NCORES)),
                               trace=trace, tmpdir=tmpdir)
    r0 = res.results[0]
    q = np.stack([res.results[i]["out_q"] for i in range(NCORES)])
    ind = np.stack([
        res.results[i]["out_ind"].transpose(0, 2, 1).reshape(NLOC, G)
        for i in range(NCORES)
    ]).astype(np.int32)
    commit = np.float32(r0["out_cl"][0, 0])
    ncs = r0["out_ncs"].reshape(G, CT, 128).reshape(G, C)
    nea = r0["out_nea"]
    ne = r0["out_ne"]
    return (q, ind, commit, ncs, nea, ne), res


def kernel(x, embed, cluster_size, embed_avg):
    out, _ = run(x, embed, cluster_size, embed_avg, trace=False)
    return out


# revision 7
# speedup vs baseline: 1.0105x; 1.0018x over previous
"""Grouped VQ (EMA codebook) on Trainium2 — 8-core data-parallel SPMD.

Hardcoded problem: x [8,2048,512] f32, embed [4,2048,128], cluster_size
[4,2048], embed_avg [4,2048,128]; G=4, C=2048, dg=128, N=16384.
Core k owns tokens of x[k] (2048). Codebooks replicated; bins/embed_sum
and the commit-loss partial are AllReduced; EMA update replicated.

Per core:
  scores[n,c] = f.(2E)^T - ||E||^2 via one fp32 PE matmul per 512-chunk
  plus a K=2 fp16 matmul folding -(e2_hi+e2_lo) into the same PSUM bank.
  argmax: nc.vector.max/max_index directly on PSUM, over two 1024 halves.
  embed_sum/bins and the quantize gather are one-hot bf16 matmuls
  (x split hi/lo bf16 so sums stay fp32-accurate; E hi/lo for gather).
"""
import sys
sys.path.insert(0, '/opt/trn_rl_repo')
import numpy as np

from concourse import bacc, tile
import concourse.mybir as mybir
from concourse.bass_utils import run_bass_kernel_spmd

F32 = mybir.dt.float32
BF16 = mybir.dt.bfloat16
FP16 = mybir.dt.float16
U32 = mybir.dt.uint32
I32 = mybir.dt.int32
AL = mybir.AluOpType
AF = mybir.ActivationFunctionType

G, C, DG = 4, 2048, 128
NLOC = 2048
NT = NLOC // 128       # 16
CT = C // 128          # 16
GT = G * CT            # 64
NCORES = 8
DECAY = 0.8
EPS = 1e-5

_CACHE = {}


def _build():
    nc = bacc.Bacc("TRN2", target_bir_lowering=False, debug=False,
                   num_devices=NCORES)

    in_x = nc.dram_tensor("in_x", [NLOC, 512], F32, kind="ExternalInput").ap()
    in_e = nc.dram_tensor("in_e", [G, C, DG], F32, kind="ExternalInput").ap()
    in_cs = nc.dram_tensor("in_cs", [128, GT], F32, kind="ExternalInput").ap()
    in_ea = nc.dram_tensor("in_ea", [G, C, DG], F32, kind="ExternalInput").ap()

    out_q = nc.dram_tensor("out_q", [NLOC, 512], F32, kind="ExternalOutput").ap()
    out_ind = nc.dram_tensor("out_ind", [NT, G, 128], I32,
                             kind="ExternalOutput").ap()
    out_ncs = nc.dram_tensor("out_ncs", [GT, 128], F32,
                             kind="ExternalOutput").ap()
    out_nea = nc.dram_tensor("out_nea", [G, C, DG], F32,
                             kind="ExternalOutput").ap()
    out_ne = nc.dram_tensor("out_ne", [G, C, DG], F32,
                            kind="ExternalOutput").ap()
    out_cl = nc.dram_tensor("out_cl", [1, 1], F32, kind="ExternalOutput").ap()

    ar_es_in = [nc.dram_tensor(f"ar_es_in{g}", [CT * 128, DG], F32)
                for g in range(G)]
    ar_es_out = [nc.dram_tensor(f"ar_es_out{g}", [CT * 128, DG], F32,
                                addr_space="Shared") for g in range(G)]
    ar_b_in = nc.dram_tensor("ar_b_in", [128, GT + 1], F32)
    ar_b_out = nc.dram_tensor("ar_b_out", [128, GT + 1], F32,
                              addr_space="Shared")

    id128f = nc.inline_tensor(np.eye(128, dtype=np.float32), name="id128f")
    iota16 = nc.inline_tensor(
        np.tile(np.arange(C, dtype=np.float16), (128, 1)), name="iota16")
    codeio = nc.inline_tensor(
        (np.arange(128, dtype=np.float32)[:, None]
         + 128.0 * np.arange(CT, dtype=np.float32)[None, :]), name="codeio")
    ones2h = nc.inline_tensor(np.ones((2, 128), np.float16), name="ones2h")
    ones1h = nc.inline_tensor(np.ones((1, 128), np.float16), name="ones1h")
    ones1fr = nc.inline_tensor(np.ones((1, 128), np.float32), name="ones1fr")
    ones1fc = nc.inline_tensor(np.ones((128, 1), np.float32), name="ones1fc")

    with tile.TileContext(nc) as tc:
        with (
            tc.tile_pool(name="cst", bufs=1) as cst,
            tc.tile_pool(name="big", bufs=1) as big,
            tc.tile_pool(name="grp", bufs=1) as grp,
            tc.tile_pool(name="grp2", bufs=2) as grp2,
            tc.tile_pool(name="wk", bufs=2) as wk,
            tc.tile_pool(name="psA", bufs=2, space="PSUM") as psA,
            tc.tile_pool(name="psB", bufs=1, space="PSUM") as psB,
            tc.tile_pool(name="psC", bufs=1, space="PSUM") as psC,
            tc.tile_pool(name="psM", bufs=2, space="PSUM") as psM,
        ):
            c_idf = cst.tile([128, 128], F32)
            nc.sync.dma_start(c_idf[:], id128f.ap())
            c_iota = cst.tile([128, C], FP16)
            nc.sync.dma_start(c_iota[:], iota16.ap())
            c_cio = cst.tile([128, CT], F32)
            nc.sync.dma_start(c_cio[:], codeio.ap())
            c_o2h = cst.tile([2, 128], FP16)
            nc.sync.dma_start(c_o2h[:], ones2h.ap())
            c_o1h = cst.tile([1, 128], FP16)
            nc.sync.dma_start(c_o1h[:], ones1h.ap())
            c_o1fr = cst.tile([1, 128], F32)
            nc.sync.dma_start(c_o1fr[:], ones1fr.ap())
            c_o1fc = cst.tile([128, 1], F32)
            nc.sync.dma_start(c_o1fc[:], ones1fc.ap())

            t_x = big.tile([128, NT, 512], F32)
            for j in range(NT):
                nc.sync.dma_start(t_x[:, j, :], in_x[j * 128:(j + 1) * 128, :])

            t_ind = big.tile([128, G, NT], F32)        # winning code ids
            t_bins = big.tile([128, GT + 1], F32)      # bins + commit partial
            t_cp = big.tile([128, G * NT], F32)        # commit partials

            def prep_group(g):
                t_eg = grp2.tile([128, CT, DG], F32, tag="eg")
                for t in range(CT):
                    nc.sync.dma_start(t_eg[:, t, :],
                                      in_e[g, t * 128:(t + 1) * 128, :])
                # gather rhs [E_hi | E_lo] bf16
                t_ehl = grp.tile([128, CT, 2 * DG], BF16, tag="ehl")
                for t in range(CT):
                    nc.vector.tensor_copy(t_ehl[:, t, 0:DG], t_eg[:, t, :])
                    nc.vector.tensor_tensor(t_ehl[:, t, DG:2 * DG],
                                            t_eg[:, t, :], t_ehl[:, t, 0:DG],
                                            AL.subtract)
                # e2 columns + negate
                t_e2c = grp2.tile([128, CT], F32, tag="e2c")
                for t in range(CT):
                    sq = wk.tile([128, DG], F32, tag="sq")
                    nc.scalar.activation(sq[:], t_eg[:, t, :], AF.Square,
                                         accum_out=t_e2c[:, t:t + 1])
                t_e2n = grp2.tile([128, CT], F32, tag="e2n")
                nc.vector.tensor_scalar(t_e2n[:], t_e2c[:], -1.0, None, AL.mult)
                # transpose -> rows, fp16 hi/lo, flatten to [2, C]
                pm = psM.tile([128, 128], F32, tag="m")
                nc.tensor.transpose(pm[0:CT, :], t_e2n[:], c_idf[:])
                t_e2nr = grp2.tile([CT, 128], F32, tag="e2nr")
                nc.scalar.copy(t_e2nr[:], pm[0:CT, :])
                t_e2h = grp2.tile([CT, 128], FP16, tag="e2h")
                t_e2l = grp2.tile([CT, 128], FP16, tag="e2l")
                nc.vector.tensor_copy(t_e2h[:], t_e2nr[:])
                nc.vector.tensor_tensor(t_e2l[:], t_e2nr[:], t_e2h[:],
                                        AL.subtract)
                t_e2r = grp2.tile([2, C], FP16, tag="e2r")
                for t in range(CT):
                    nc.sync.dma_start(t_e2r[0:1, t * 128:(t + 1) * 128],
                                      t_e2h[t:t + 1, :])
                    nc.sync.dma_start(t_e2r[1:2, t * 128:(t + 1) * 128],
                                      t_e2l[t:t + 1, :])
                # transposes: eT2 = (2E)^T fp32 ; xT_g fp32
                t_eT2 = grp2.tile([128, C], F32, tag="eT2")
                for t in range(CT):
                    pm = psM.tile([128, 128], F32, tag="m")
                    nc.tensor.transpose(pm[:], t_eg[:, t, :], c_idf[:])
                    nc.scalar.mul(t_eT2[:, t * 128:(t + 1) * 128], pm[:], 2.0)
                t_xT = grp2.tile([128, NLOC], F32, tag="xT")
                for j in range(NT):
                    pm = psM.tile([128, 128], F32, tag="m")
                    nc.tensor.transpose(pm[:], t_x[:, j, g * 128:(g + 1) * 128],
                                        c_idf[:])
                    nc.scalar.copy(t_xT[:, j * 128:(j + 1) * 128], pm[:])
                # embed_sum rhs [x_hi | x_lo | 1] bf16 per token tile
                t_rhs = grp.tile([128, NT, 2 * DG + 1], BF16, tag="rhs")
                nc.vector.memset(t_rhs[:, :, 2 * DG], 1.0)
                for j in range(NT):
                    nc.vector.tensor_copy(t_rhs[:, j, 0:DG],
                                          t_x[:, j, g * 128:(g + 1) * 128])
                    nc.vector.tensor_tensor(t_rhs[:, j, DG:2 * DG],
                                            t_x[:, j, g * 128:(g + 1) * 128],
                                            t_rhs[:, j, 0:DG], AL.subtract)
                return t_ehl, t_e2r, t_eT2, t_xT, t_rhs

            nxt = prep_group(0)
            for g in range(G):
                t_ehl, t_e2r, t_eT2, t_xT, t_rhs = nxt

                # ---- dist + argmax ----
                t_mx = grp.tile([128, NT, 2, 8], F32, tag="mx")
                t_ix = grp.tile([128, NT, 2, 8], U32, tag="ix")
                for j in range(NT):
                    for h in range(2):
                        pd = psA.tile([128, 1024], F32, tag="dist")
                        for q in range(2):
                            cc = h * 2 + q
                            nc.tensor.matmul(
                                pd[:, q * 512:(q + 1) * 512],
                                t_xT[:, j * 128:(j + 1) * 128],
                                t_eT2[:, cc * 512:(cc + 1) * 512],
                                start=True, stop=False)
                            nc.tensor.matmul(
                                pd[:, q * 512:(q + 1) * 512],
                                c_o2h[:],
                                t_e2r[:, cc * 512:(cc + 1) * 512],
                                start=False, stop=True)
                        nc.vector.max(t_mx[:, j, h, :], pd[:])
                        nc.vector.max_index(t_ix[:, j, h, :], t_mx[:, j, h, :],
                                            pd[:])
                # combine halves (vectorized over NT)
                t_sel = wk.tile([128, NT], F32, tag="sel")
                nc.vector.tensor_tensor(t_sel[:], t_mx[:, :, 0, 0],
                                        t_mx[:, :, 1, 0], AL.is_ge)
                t_i0 = wk.tile([128, NT], F32, tag="i0")
                nc.vector.tensor_copy(t_i0[:], t_ix[:, :, 0, 0])
                t_i1 = wk.tile([128, NT], F32, tag="i1")
                nc.vector.tensor_copy(t_i1[:], t_ix[:, :, 1, 0])
                nc.vector.tensor_scalar(t_i1[:], t_i1[:], 1024.0, None, AL.add)
                t_d = wk.tile([128, NT], F32, tag="idd")
                nc.vector.tensor_tensor(t_d[:], t_i0[:], t_i1[:], AL.subtract)
                nc.vector.tensor_tensor(t_d[:], t_sel[:], t_d[:], AL.mult)
                nc.vector.tensor_tensor(t_ind[:, g, :], t_i1[:], t_d[:], AL.add)

                if g + 1 < G:
                    nxt = prep_group(g + 1)

                # ---- ind rows: transpose, int32 out, fp16 row, broadcast ----
                pm = psM.tile([128, 128], F32, tag="m")
                nc.tensor.transpose(pm[0:NT, :], t_ind[:, g, :], c_idf[:])
                t_indi = wk.tile([NT, 128], I32, tag="indi")
                nc.scalar.copy(t_indi[:], pm[0:NT, :])
                nc.sync.dma_start(out_ind[:, g, :], t_indi[:])
                t_indr = wk.tile([NT, 128], FP16, tag="indr")
                nc.scalar.copy(t_indr[:], pm[0:NT, :])
                t_irow = grp.tile([1, NLOC], FP16, tag="irow")
                for j in range(NT):
                    nc.sync.dma_start(t_irow[0:1, j * 128:(j + 1) * 128],
                                      t_indr[j:j + 1, :])
                t_ibc = grp.tile([128, NLOC], FP16, tag="ibc")
                for j in range(NT):
                    pm = psM.tile([128, 128], F32, tag="m")
                    nc.tensor.matmul(pm[:], c_o1h[:],
                                     t_irow[0:1, j * 128:(j + 1) * 128],
                                     start=True, stop=True)
                    nc.scalar.copy(t_ibc[:, j * 128:(j + 1) * 128], pm[:])

                # ---- onehot [n|c] -> embed_sum + bins ----
                t_oh = big.tile([128, NT, C], BF16, tag="oh")
                for j in range(NT):
                    nc.vector.tensor_scalar(t_oh[:, j, :], c_iota[:],
                                            t_ind[:, g, j:j + 1], None,
                                            AL.is_equal)
                for t in range(CT):
                    pe = psB.tile([128, 2 * DG + 1], F32, tag="es")
                    for j in range(NT):
                        nc.tensor.matmul(pe[:],
                                         t_oh[:, j, t * 128:(t + 1) * 128],
                                         t_rhs[:, j, :],
                                         start=(j == 0), stop=(j == NT - 1))
                    esh = wk.tile([128, DG], F32, tag="esh")
                    nc.scalar.copy(esh[:], pe[:, 0:DG])
                    es = wk.tile([128, DG], F32, tag="esd")
                    nc.vector.tensor_tensor(es[:], pe[:, DG:2 * DG], esh[:],
                                            AL.add)
                    nc.sync.dma_start(
                        ar_es_in[g].ap()[t * 128:(t + 1) * 128, :], es[:])
                    nc.scalar.copy(t_bins[:, g * CT + t:g * CT + t + 1],
                                   pe[:, 2 * DG:2 * DG + 1])

                nc.gpsimd.collective_compute(
                    "AllReduce", AL.add,
                    replica_groups=[list(range(NCORES))],
                    ins=[ar_es_in[g].ap().opt()],
                    outs=[ar_es_out[g].ap().opt()])

                # ---- onehotT [c|n] -> gather quantize ----
                t_ohT = big.tile([128, CT, NLOC], BF16, tag="oh")
                for t in range(CT):
                    nc.vector.tensor_scalar(t_ohT[:, t, :], t_ibc[:],
                                            c_cio[:, t:t + 1], None,
                                            AL.is_equal)
                for j in range(NT):
                    pq = psC.tile([128, 2 * DG], F32, tag="q")
                    for t in range(CT):
                        nc.tensor.matmul(pq[:],
                                         t_ohT[:, t, j * 128:(j + 1) * 128],
                                         t_ehl[:, t, :],
                                         start=(t == 0), stop=(t == CT - 1))
                    tqh = wk.tile([128, DG], F32, tag="tqh")
                    nc.scalar.copy(tqh[:], pq[:, 0:DG])
                    tq = wk.tile([128, DG], F32, tag="tq")
                    nc.vector.tensor_tensor(tq[:], pq[:, DG:2 * DG], tqh[:],
                                            AL.add)
                    nc.sync.dma_start(
                        out_q[j * 128:(j + 1) * 128, g * 128:(g + 1) * 128],
                        tq[:])
                    td = wk.tile([128, DG], F32, tag="td")
                    nc.vector.tensor_tensor(td[:], tq[:],
                                            t_x[:, j, g * 128:(g + 1) * 128],
                                            AL.subtract)
                    nc.scalar.activation(td[:], td[:], AF.Square,
                                         accum_out=t_cp[:, g * NT + j:g * NT + j + 1])

            # ---- commit partial col + AllReduce ----
            nc.vector.tensor_reduce(t_bins[:, GT:GT + 1], t_cp[:],
                                    mybir.AxisListType.X, AL.add)
            nc.sync.dma_start(ar_b_in.ap(), t_bins[:])
            nc.gpsimd.collective_compute(
                "AllReduce", AL.add, replica_groups=[list(range(NCORES))],
                ins=[ar_b_in.ap().opt()], outs=[ar_b_out.ap().opt()])

            # ---- EMA update (replicated on every core) ----
            t_bar = big.tile([128, GT + 1], F32, tag="bar")
            nc.sync.dma_start(t_bar[:], ar_b_out.ap())
            t_cs = big.tile([128, GT], F32, tag="cs")
            nc.sync.dma_start(t_cs[:], in_cs[:])
            t_ncs = big.tile([128, GT], F32, tag="ncs")
            t_tmp = wk.tile([128, GT], F32, tag="tmp")
            nc.vector.tensor_scalar(t_tmp[:], t_cs[:], DECAY, None, AL.mult)
            nc.vector.scalar_tensor_tensor(t_ncs[:], t_bar[:, 0:GT],
                                           1.0 - DECAY, t_tmp[:],
                                           AL.mult, AL.add)
            # partition sums (ncs totals + global commit)
            pm = psM.tile([128, 128], F32, tag="m")
            nc.tensor.matmul(pm[0:1, 0:GT], c_o1fc[:], t_ncs[:],
                             start=True, stop=True)
            nc.tensor.matmul(pm[0:1, GT:GT + 1], c_o1fc[:],
                             t_bar[:, GT:GT + 1], start=True, stop=True)
            s_su = wk.tile([1, GT + 1], F32, tag="ssu")
            nc.scalar.copy(s_su[:], pm[0:1, 0:GT + 1])
            s_cl = wk.tile([1, 1], F32, tag="scl")
            nc.vector.tensor_scalar(s_cl[:], s_su[:, GT:GT + 1],
                                    1.0 / (NCORES * NLOC * 512), None, AL.mult)
            nc.sync.dma_start(out_cl[:], s_cl[:])
            t_tot = wk.tile([1, G], F32, tag="tot")
            for g in range(G):
                nc.vector.tensor_reduce(t_tot[:, g:g + 1],
                                        s_su[:, g * CT:(g + 1) * CT],
                                        mybir.AxisListType.X, AL.add)
            t_totx = wk.tile([1, GT], F32, tag="totx")
            for g in range(G):
                nc.vector.tensor_scalar(t_totx[:, g * CT:(g + 1) * CT],
                                        c_o1fr[0:1, 0:CT], t_tot[:, g:g + 1],
                                        None, AL.mult)
            pm2 = psM.tile([128, 128], F32, tag="m")
            nc.tensor.matmul(pm2[:, 0:GT], c_o1fr[:], t_totx[:],
                             start=True, stop=True)
            t_totb = wk.tile([128, GT], F32, tag="totb")
            nc.scalar.copy(t_totb[:], pm2[:, 0:GT])
            # smoothed + reciprocal
            t_sm = wk.tile([128, GT], F32, tag="sm")
            nc.vector.tensor_scalar(t_sm[:], t_ncs[:], EPS, None, AL.add)
            t_den = wk.tile([128, GT], F32, tag="den")
            nc.vector.tensor_scalar(t_den[:], t_totb[:], C * EPS, None, AL.add)
            t_rden = wk.tile([128, GT], F32, tag="rden")
            nc.vector.reciprocal(t_rden[:], t_den[:])
            nc.vector.tensor_tensor(t_sm[:], t_sm[:], t_rden[:], AL.mult)
            nc.vector.tensor_tensor(t_sm[:], t_sm[:], t_totb[:], AL.mult)
            t_rsm = wk.tile([128, GT], F32, tag="rsm")
            nc.vector.reciprocal(t_rsm[:], t_sm[:])
            # ncs out (transposed)
            pm3 = psM.tile([128, 128], F32, tag="m")
            nc.tensor.transpose(pm3[0:GT, :], t_ncs[:], c_idf[:])
            s_nc = wk.tile([GT, 128], F32, tag="snc")
            nc.scalar.copy(s_nc[:], pm3[0:GT, :])
            nc.sync.dma_start(out_ncs[:], s_nc[:])
            # nea / ne
            for g in range(G):
                for t in range(CT):
                    ea = wk.tile([128, DG], F32, tag="ea")
                    nc.sync.dma_start(ea[:], in_ea[g, t * 128:(t + 1) * 128, :])
                    esr = wk.tile([128, DG], F32, tag="esr")
                    nc.sync.dma_start(
                        esr[:], ar_es_out[g].ap()[t * 128:(t + 1) * 128, :])
                    nea = wk.tile([128, DG], F32, tag="nea")
                    nc.vector.tensor_scalar(nea[:], ea[:], DECAY, None, AL.mult)
                    nc.vector.scalar_tensor_tensor(nea[:], esr[:], 1.0 - DECAY,
                                                   nea[:], AL.mult, AL.add)
                    nc.sync.dma_start(out_nea[g, t * 128:(t + 1) * 128, :],
                                      nea[:])
                    ne = wk.tile([128, DG], F32, tag="ne")
                    nc.vector.tensor_scalar(
                        ne[:], nea[:], t_rsm[:, g * CT + t:g * CT + t + 1],
                        None, AL.mult)
                    nc.sync.dma_start(out_ne[g, t * 128:(t + 1) * 128, :],
                                      ne[:])

    nc.compile()
    return nc


def _get_nc():
    if "nc" not in _CACHE:
        _CACHE["nc"] = _build()
    return _CACHE["nc"]


def run(x, embed, cluster_size, embed_avg, trace=False, tmpdir=None):
    nc = _get_nc()
    x = np.ascontiguousarray(np.asarray(x, dtype=np.float32))
    embed = np.ascontiguousarray(np.asarray(embed, dtype=np.float32))
    cluster_size = np.asarray(cluster_size, dtype=np.float32)
    embed_avg = np.ascontiguousarray(np.asarray(embed_avg, dtype=np.float32))
    # cluster_size -> [128 c-local, 64 (g,t)] layout
    cs_t = np.ascontiguousarray(
        cluster_size.reshape(G, CT, 128).transpose(2, 0, 1).reshape(128, GT))
    in_maps = [
        {"in_x": np.ascontiguousarray(x[i]), "in_e": embed, "in_cs": cs_t,
         "in_ea": embed_avg}
        for i in range(NCORES)
    ]
    res = run_bass_kernel_spmd(nc, in_maps, list(range(NCORES)),
                               trace=trace, tmpdir=tmpdir)
    r0 = res.results[0]
    q = np.stack([res.results[i]["out_q"] for i in range(NCORES)])
    ind = np.stack([
        res.results[i]["out_ind"].transpose(0, 2, 1).reshape(NLOC, G)
        for i in range(NCORES)
    ]).astype(np.int32)
    commit = np.float32(r0["out_cl"][0, 0])
    ncs = r0["out_ncs"].reshape(G, CT, 128).reshape(G, C)
    nea = r0["out_nea"]
    ne = r0["out_ne"]
    return (q, ind, commit, ncs, nea, ne), res


def kernel(x, embed, cluster_size, embed_avg):
    out, _ = run(x, embed, cluster_size, embed_avg, trace=False)
    return out
